# revision 1
# baseline (speedup 1.0000x reference)
"""Trainium2 Bass kernel for the 5x5 circular-padded conv
   y = conv5x5_circular(x[16,64,384,768], w[64,64,5,5]) + b.

Self-contained: shards the batch dim T=16 across 8 NeuronCores (2 images
per core), runs one SPMD Bass/Tile program, gathers the full output.

Per-core kernel: implicit GEMM over filter taps in float32r (1 cyc/row
on TensorE at even free dim >= 256, ~1.5e-4 rel err). fp32r forbids PE
column tiling (PSUM out must start at partition 0), so the 128-wide
array is filled via:
 - 2x row tiling: position T0 (SBUF partitions 0-63, x copy1) and T8
   (partitions 64-127, x copy2) stream rhs concurrently, each K=64.
 - M=128 output-shift packing: lhsT [64ci,128]: cols 0-63 = tap (dy,dx),
   cols 64-127 = tap (dy,dx+1); PSUM row 64+co, col n is a partial of
   output pixel n-1, merged with a +1 column shift (DVE cross-partition
   PSUM read). Taps (dy,4) zero the upper weight half (uniform 64x128
   tiling mode, no PE mode-switch drains).
Input is circularly padded on the host -> xp [2, 64, 388, 774].
"""

import numpy as np

import concourse.mybir as mybir
from concourse.tile import TileContext
from concourse import bacc
from concourse import bass_utils

F32 = mybir.dt.float32
F32R = mybir.dt.float32r
AFT = mybir.ActivationFunctionType

SLOT_DX0 = [0, 2, 4]
N_CORES = 8
T, C, H, W = 16, 64, 384, 768

_cache = {}


def _build_conv(T_loc, H, W, R=16):
    Hp, Wp = H + 4, W + 6
    Wh = W // 2
    Ns = Wh + 2
    nbands = H // R

    nc = bacc.Bacc("TRN2", target_bir_lowering=False, debug=False)
    xp = nc.dram_tensor("xp", [T_loc, C, Hp, Wp], F32R, kind="ExternalInput").ap()
    wd = nc.dram_tensor("wd", [128, 15 * 128], F32R, kind="ExternalInput").ap()
    bd = nc.dram_tensor("bd", [64, 1], F32, kind="ExternalInput").ap()
    y = nc.dram_tensor("y", [T_loc, C, H, W], F32, kind="ExternalOutput").ap()

    with TileContext(nc) as tc:
        with (
            tc.tile_pool(name="const", bufs=1) as cpool,
            tc.tile_pool(name="xband", bufs=2) as xpool,
            tc.tile_pool(name="yrow", bufs=4) as ypool,
            tc.tile_pool(name="psum", bufs=3, space="PSUM") as ppool,
        ):
            wsb = cpool.tile([128, 15 * 128], F32R)
            nc.sync.dma_start(out=wsb, in_=wd)
            bsb = cpool.tile([64, 1], F32)
            nc.sync.dma_start(out=bsb, in_=bd)

            for t in range(T_loc):
                for band in range(nbands):
                    r0 = band * R
                    xb = xpool.tile([128, R + 4, Wp], F32R)
                    nc.sync.dma_start(out=xb[0:64], in_=xp[t, :, r0 : r0 + R + 4, :])
                    nc.sync.dma_start(out=xb[64:128], in_=xb[0:64])
                    for h in range(R):
                        for wb in (0, Wh):
                            ps1 = ppool.tile([128, Ns], F32, tag="ps1")
                            ps2 = ppool.tile([128, Ns], F32, tag="ps2")
                            for k in range(8):
                                for pos, s in ((0, k), (1, 8 + k)):
                                    if pos and s > 14:
                                        continue
                                    dy, kk = divmod(s, 3)
                                    dx0 = SLOT_DX0[kk]
                                    lo, hi = (0, 64) if pos == 0 else (64, 128)
                                    ps = ps1 if pos == 0 else ps2
                                    nc.tensor.matmul(
                                        ps,
                                        wsb[lo:hi, s * 128 : (s + 1) * 128],
                                        xb[lo:hi, h + dy, wb + dx0 : wb + dx0 + Ns],
                                        start=(s == 0 or s == 8),
                                        stop=(s == 7 or s == 14),
                                    )
                            t1 = ypool.tile([64, Wh], F32, tag="t1")
                            t2 = ypool.tile([64, Wh], F32, tag="t2")
                            nc.scalar.activation(t1, ps1[0:64, 0:Wh], AFT.Identity, bias=bsb)
                            nc.scalar.activation(t2, ps2[0:64, 0:Wh], AFT.Identity, bias=0.0)
                            nc.vector.tensor_add(out=t1, in0=t1, in1=ps1[64:128, 1 : Wh + 1])
                            nc.vector.tensor_add(out=t2, in0=t2, in1=ps2[64:128, 1 : Wh + 1])
                            nc.vector.tensor_add(out=t1, in0=t1, in1=t2)
                            nc.sync.dma_start(out=y[t, :, r0 + h, wb : wb + Wh], in_=t1)
    nc.compile()
    return nc


def _make_wd(w):
    wd = np.zeros((64, 15, 128), dtype=np.float32)
    for dy in range(5):
        for k, dx0 in enumerate(SLOT_DX0):
            s = dy * 3 + k
            wd[:, s, 0:64] = w[:, :, dy, dx0].T
            if dx0 + 1 < 5:
                wd[:, s, 64:128] = w[:, :, dy, dx0 + 1].T
    wd = wd.reshape(64, 15 * 128)
    return np.ascontiguousarray(np.concatenate([wd, wd], axis=0))


def kernel(x, w, b):
    x = np.asarray(x, dtype=np.float32)
    w = np.asarray(w, dtype=np.float32)
    b = np.asarray(b, dtype=np.float32)
    assert x.shape == (T, C, H, W), x.shape

    T_loc = T // N_CORES
    if "nc" not in _cache:
        _cache["nc"] = _build_conv(T_loc, H, W)
    nc = _cache["nc"]

    xpad = np.pad(x, ((0, 0), (0, 0), (2, 2), (2, 4)), mode="wrap")
    wd = _make_wd(w)
    bd = b.reshape(64, 1).copy()
    in_maps = [
        {
            "xp": np.ascontiguousarray(xpad[c * T_loc : (c + 1) * T_loc]),
            "wd": wd,
            "bd": bd,
        }
        for c in range(N_CORES)
    ]
    res = bass_utils.run_bass_kernel_spmd(nc, in_maps, core_ids=list(range(N_CORES)))
    return np.concatenate([res.results[c]["y"] for c in range(N_CORES)], axis=0)


# revision 2
# speedup vs baseline: 1.1671x; 1.1671x over previous
"""Trainium2 Bass kernel for the 5x5 circular-padded conv
   y = conv5x5_circular(x[16,64,384,768], w[64,64,5,5]) + b.

Self-contained: shards the batch dim T=16 across 8 NeuronCores (2 images
per core), runs one SPMD Bass/Tile program, gathers the full output.

Per-core kernel: implicit GEMM over filter taps in float32r (1 cyc/row
on TensorE at even free dim >= 256, ~1.5e-4 rel err). fp32r forbids PE
column tiling (PSUM out must start at partition 0), so the 128-wide
array is filled via:
 - 2x row tiling: position T0 (SBUF partitions 0-63, x copy1) and T8
   (partitions 64-127, x copy2) stream rhs concurrently, each K=64.
 - M=128 output-shift packing: lhsT [64ci,128]: cols 0-63 = tap (dy,dx),
   cols 64-127 = tap (dy,dx+1); PSUM row 64+co, col n is a partial of
   output pixel n-1, merged with a +1 column shift (DVE cross-partition
   PSUM read). Taps (dy,4) zero the upper weight half (uniform 64x128
   tiling mode, no PE mode-switch drains).
Input is circularly padded on the host -> xp [2, 64, 388, 774].
"""

import numpy as np

import concourse.mybir as mybir
from concourse.tile import TileContext
from concourse import bacc
from concourse import bass_utils

F32 = mybir.dt.float32
F32R = mybir.dt.float32r
AFT = mybir.ActivationFunctionType

SLOT_DX0 = [0, 2, 4]
N_CORES = 8
T, C, H, W = 16, 64, 384, 768

_cache = {}


def _build_conv(T_loc, H, W, R=16):
    Hp, Wp = H + 4, W + 6
    Wh = W // 2
    Ns = Wh + 2
    nbands = H // R

    nc = bacc.Bacc("TRN2", target_bir_lowering=False, debug=False)
    xp = nc.dram_tensor("xp", [T_loc, C, Hp, Wp], F32R, kind="ExternalInput").ap()
    wd = nc.dram_tensor("wd", [128, 15 * 128], F32R, kind="ExternalInput").ap()
    bd = nc.dram_tensor("bd", [64, 1], F32, kind="ExternalInput").ap()
    y = nc.dram_tensor("y", [T_loc, C, H, W], F32, kind="ExternalOutput").ap()

    with TileContext(nc) as tc:
        with (
            tc.tile_pool(name="const", bufs=1) as cpool,
            tc.tile_pool(name="xband", bufs=2) as xpool,
            tc.tile_pool(name="yrow", bufs=6) as ypool,
            tc.tile_pool(name="psum", bufs=4, space="PSUM") as ppool,
        ):
            wsb = cpool.tile([128, 15 * 128], F32R)
            nc.sync.dma_start(out=wsb, in_=wd)
            bsb = cpool.tile([64, 1], F32)
            nc.sync.dma_start(out=bsb, in_=bd)

            for t in range(T_loc):
                for band in range(nbands):
                    r0 = band * R
                    xb = xpool.tile([128, R + 4, Wp], F32R)
                    nc.sync.dma_start(out=xb[0:64], in_=xp[t, :, r0 : r0 + R + 4, :])
                    nc.sync.dma_start(out=xb[64:128], in_=xb[0:64])
                    for h in range(R):
                        for wb in (0, Wh):
                            ps1 = ppool.tile([128, Ns], F32, tag="ps1")
                            ps2 = ppool.tile([128, Ns], F32, tag="ps2")
                            # Perfect T8/T0 alternation (incl. across tile
                            # boundaries) so the two row-group streams always
                            # overlap: T8 gets slots {14, 7..13}, T0 0..6.
                            seq = [(1, 14)]
                            for k in range(7):
                                seq.append((0, k))
                                seq.append((1, 7 + k))
                            for pos, s in seq:
                                dy, kk = divmod(s, 3)
                                dx0 = SLOT_DX0[kk]
                                lo, hi = (0, 64) if pos == 0 else (64, 128)
                                ps = ps1 if pos == 0 else ps2
                                nc.tensor.matmul(
                                    ps,
                                    wsb[lo:hi, s * 128 : (s + 1) * 128],
                                    xb[lo:hi, h + dy, wb + dx0 : wb + dx0 + Ns],
                                    start=(s == 0 or s == 14),
                                    stop=(s == 6 or s == 13),
                                )
                            t1 = ypool.tile([64, Wh], F32, tag="t1")
                            t2 = ypool.tile([64, Wh], F32, tag="t2")
                            nc.scalar.activation(t1, ps1[0:64, 0:Wh], AFT.Identity, bias=bsb)
                            nc.scalar.activation(t2, ps2[0:64, 0:Wh], AFT.Identity, bias=0.0)
                            nc.vector.tensor_add(out=t1, in0=t1, in1=ps1[64:128, 1 : Wh + 1])
                            nc.vector.tensor_add(out=t2, in0=t2, in1=ps2[64:128, 1 : Wh + 1])
                            nc.gpsimd.tensor_add(out=t1, in0=t1, in1=t2)
                            nc.sync.dma_start(out=y[t, :, r0 + h, wb : wb + Wh], in_=t1)
    nc.compile()
    return nc


def _make_wd(w):
    wd = np.zeros((64, 15, 128), dtype=np.float32)
    for dy in range(5):
        for k, dx0 in enumerate(SLOT_DX0):
            s = dy * 3 + k
            wd[:, s, 0:64] = w[:, :, dy, dx0].T
            if dx0 + 1 < 5:
                wd[:, s, 64:128] = w[:, :, dy, dx0 + 1].T
    wd = wd.reshape(64, 15 * 128)
    return np.ascontiguousarray(np.concatenate([wd, wd], axis=0))


def kernel(x, w, b):
    x = np.asarray(x, dtype=np.float32)
    w = np.asarray(w, dtype=np.float32)
    b = np.asarray(b, dtype=np.float32)
    assert x.shape == (T, C, H, W), x.shape

    T_loc = T // N_CORES
    if "nc" not in _cache:
        _cache["nc"] = _build_conv(T_loc, H, W)
    nc = _cache["nc"]

    xpad = np.pad(x, ((0, 0), (0, 0), (2, 2), (2, 4)), mode="wrap")
    wd = _make_wd(w)
    bd = b.reshape(64, 1).copy()
    in_maps = [
        {
            "xp": np.ascontiguousarray(xpad[c * T_loc : (c + 1) * T_loc]),
            "wd": wd,
            "bd": bd,
        }
        for c in range(N_CORES)
    ]
    res = bass_utils.run_bass_kernel_spmd(nc, in_maps, core_ids=list(range(N_CORES)))
    return np.concatenate([res.results[c]["y"] for c in range(N_CORES)], axis=0)
